# revision 106
# baseline (speedup 1.0000x reference)
"""Trainium2 Bass kernel for LBLHighwayBiLm.

Reference computation (per layer l of L=2, on [B=32, S=512, H=512] input):
  fwd/bwd depthwise window conv (5 taps, scalar weight per tap) with learned
  boundary pads, then NHW=2 highway layers per direction:
      proj = x @ W^T + b;  nl = relu(proj[:H]);  g = sigmoid(proj[H:])
      x = g * x + (1 - g) * nl
  output[l] = concat([f_out, b_out], -1)

Strategy: data-parallel over batch (4 per core x 8 cores), float16 state
end-to-end (rel err ~1.3e-3 vs the 2e-2 gate; f16 matmul costs the same
1 cycle/row as fp32r on PE but unlocks the DVE 2x/4x perf modes and halves
DMA bytes). The tensor engine runs ONLY the highway GEMMs (218.5us/core,
the hard roofline: 524288 psum rows x 0.4167ns); 96 warmup matmuls ramp
its p-state and keep it busy while the DVE runs the layer-0 convs. The
5-tap conv runs on DVE as tensor_scalar_mul (4x mode) + tensor_add (2x)
chains, with the k=0 scaled copy on the (startup-idle) scalar engine for
layer 0. Highway combines (3 tensor-tensor ops) are split between DVE and
the otherwise-idle GPSIMD/Pool engine via a tuned per-(layer, i, dir)
table; a dummy sigmoid preloads the relu+sigmoid activation table. Work
is pipelined as 4 independent chains (2 directions x 2 batch halves);
each direction rotates THREE state buffers so the next layer's conv
(emitted right after each i=1 phase) writes a dead buffer instead of
racing the i=1 GEMM's reads. Weights stay one layer resident (32KB) and
are re-streamed mid-run, freeing SBUF for 16-deep nl/g evacuation rings
(shallower rings starve the scalar engine behind lagging combines and
stall PE ~8us). PSUM is evacuated by the scalar engine with fused
bias+relu/sigmoid; outputs stream out per (j, b) right after each final
combine, and the last phase uses per-batch-row granularity for a short
drain tail. Cost-model timeline: 257902 ns/core (baseline 319368).
"""
import numpy as np

B, S, H, L, W, NHW = 32, 512, 512, 2, 4, 2
NCORES = 8
BL = B // NCORES          # batch per core (4)
BH = 2                    # batch per half-chain
P = 128
HB = H // P               # h blocks (4)
MB = 2 * H // P           # proj out blocks (8)
SW = S + W                # padded row width

_CACHE = {}


def _build_nc():
    import concourse.bass as bass
    import concourse.tile as tile
    from concourse import bacc, mybir

    f32 = mybir.dt.float32
    f16 = mybir.dt.float16
    AF = mybir.ActivationFunctionType

    nc = bacc.Bacc("TRN2", target_bir_lowering=False)

    xt = nc.dram_tensor("xt", [BL, H, S], f16, kind="ExternalInput")
    wt = nc.dram_tensor("wt", [L, 2, NHW, H, 2 * H], f16, kind="ExternalInput")
    padl = nc.dram_tensor("padl", [L, H, W], f16, kind="ExternalInput")
    padr = nc.dram_tensor("padr", [L, H, W], f16, kind="ExternalInput")
    hwb = nc.dram_tensor("hwb", [L, 2, NHW, P, MB], f32, kind="ExternalInput")
    ws = nc.dram_tensor("ws", [L, 2, W + 1], f32, kind="ExternalInput")
    out = nc.dram_tensor("out", [L, BL, 2 * H, S], f16, kind="ExternalOutput")

    with tile.TileContext(nc) as tc:
        with (
            tc.tile_pool(name="state", bufs=1) as state_pool,
            tc.tile_pool(name="singles", bufs=1) as singles,
            tc.tile_pool(name="evac", bufs=16) as evac_pool,
            tc.tile_pool(name="tmp", bufs=2) as tmp_pool,
            tc.tile_pool(name="ps", bufs=4, space="PSUM") as ps_pool,
        ):
            hwb_sb = singles.tile([P, L, 2, NHW, MB], f32, tag="hwb", name="hwb_sb")
            nc.sync.dma_start(
                out=hwb_sb, in_=hwb.rearrange("l d i p m -> p l d i m")
            )
            ws_sb = singles.tile([P, L, 2, W + 1], f32, tag="ws", name="ws_sb")
            wsap = ws[:]
            nc.sync.dma_start(
                out=ws_sb,
                in_=bass.AP(tensor=wsap.tensor, offset=wsap.offset,
                            ap=[[0, P]] + list(wsap.ap)),
            )

            # three state buffers per direction, rotating roles per layer:
            # conv: rot[0] -> rot[1]; i=0: rot[1] -> rot[2];
            # i=1: rot[2] -> rot[1]; conv(l+1): rot[1] -> rot[0]; then
            # rot[0] <-> rot[1].
            rot = {
                d: [state_pool.tile([P, HB, BL, SW], f16, tag=f"s{d}{k}",
                                    name=f"s{d}{k}") for k in range(3)]
                for d in range(2)
            }
            OFF = {0: W, 1: 0}       # payload offset per direction
            PADOFF = {0: 0, 1: S}    # pad-slot offset per direction

            def load_pads(l, dstbuf, d):
                # per-hb broadcast DMA: stride-0 batch dim on the dram side
                # replicates the pads into every b row (4 DMAs per (l, d))
                psrc = (padl if d == 0 else padr)[l].rearrange(
                    "(hb p) w -> p hb w", p=P
                )
                po = PADOFF[d]
                for hb in range(HB):
                    s = psrc[:, hb, :]
                    src = bass.AP(
                        tensor=s.tensor, offset=s.offset,
                        ap=[list(s.ap[0]), [0, BL], list(s.ap[1])],
                    )
                    nc.sync.dma_start(
                        out=dstbuf[:, hb, :, po:po + W], in_=src
                    )

            # small PE warmup head while the first x/ids DMAs are in flight
            warm_src = singles.tile([P, S], f16, tag="wsrc", name="wsrc")
            warm_w = singles.tile([P, P], f16, tag="ww", name="ww")
            scratch = singles.tile([P, 1], f16, tag="scr", name="scratch")
            nc.gpsimd.memset(warm_src, 0.0)
            nc.gpsimd.memset(warm_w, 0.0)
            # dummy sigmoid: forces the relu+sigmoid activation-table load
            # into the idle startup window instead of before the first real
            # sigmoid evacuation
            nc.scalar.activation(out=scratch, in_=warm_w[:, 0:1],
                                 func=AF.Sigmoid)
            # warmup junk matmuls: ramp the tensor engine p-state to full
            # clock and keep it busy while the DVE runs the layer-0 convs
            for _ in range(96):
                wps = ps_pool.tile([P, BH, S], f32, tag="ps", name="ps")
                nc.tensor.matmul(wps[:, 0, :], lhsT=warm_w, rhs=warm_src,
                                 start=True, stop=True)

            # startup DMA order matters (the cost model serializes the DMA
            # device): pads + x first in first-use order (the first convs
            # gate everything), weights after.
            for d in range(2):
                o = OFF[d]
                for h in range(2):
                    for hb in range(HB):
                        xv = xt[2 * h:2 * h + BH,
                                hb * P:(hb + 1) * P, :].rearrange("b p s -> p b s")
                        nc.sync.dma_start(
                            out=rot[d][0][:, hb, 2 * h:2 * h + BH, o:o + S],
                            in_=xv,
                        )
                        if hb == 0 and h == 0:
                            load_pads(0, rot[d][0], d)

            # highway weights: one layer resident at a time (32KB/partition
            # in f16, freeing room for deep nl/g rings); layer-1 slots are
            # re-streamed mid-run right after layer 0 stops reading them
            wt_sb = singles.tile([P, 2, NHW, HB, 2 * H], f16, tag="wt",
                                 name="wt_sb")

            def load_wt(l, d, i):
                nc.sync.dma_start(
                    out=wt_sb[:, d, i],
                    in_=wt[l, d, i].rearrange("(kb p) o -> p kb o", p=P),
                )

            for i in range(NHW):
                for d in range(2):
                    load_wt(0, d, i)

            def conv_chain(l, d, h, hb, src, dst, pool=False, k0_act=False,
                           k1_act=False):
                # payload(dst)[:, hb, bsl, t] = sum_k w[k]*src[:, hb, bsl, t+k]
                # k=0 as tensor_scalar_mul (DVE 4x mode), k>0 as mul into a
                # tmp tile (4x) + tensor_add (2x). k0_act (layer-0 only,
                # while the scalar engine is idle) moves the k=0 op to a
                # scaled copy on the scalar engine.
                o = OFF[d]
                bsl = slice(2 * h, 2 * h + BH)
                acc = dst[:, hb, bsl, o:o + S]
                eng = nc.gpsimd if pool else nc.vector
                if k0_act:
                    nc.scalar.activation(
                        out=acc, in_=src[:, hb, bsl, 0:S], func=AF.Copy,
                        scale=ws_sb[:, l, d, 0:1],
                    )
                else:
                    eng.tensor_scalar_mul(
                        acc, src[:, hb, bsl, 0:S], ws_sb[:, l, d, 0:1]
                    )
                for k in range(1, W + 1):
                    tmp = tmp_pool.tile([P, BH, S], f16,
                                        tag="ctp" if pool else "ct",
                                        name="ctmp")
                    if k1_act and k == 1:
                        nc.scalar.activation(
                            out=tmp, in_=src[:, hb, bsl, k:k + S],
                            func=AF.Copy, scale=ws_sb[:, l, d, k:k + 1],
                        )
                    else:
                        eng.tensor_scalar_mul(
                            tmp, src[:, hb, bsl, k:k + S],
                            ws_sb[:, l, d, k:k + 1]
                        )
                    eng.tensor_add(acc, acc, tmp)

            deferred_combines = []

            def gemm_phase(l, d, i, h, xin, xout, pool_js, conv_dst,
                           fine=False, defer_dve_combines=False):
                # one phase: GEMM + evac + combine for batch half h. proj
                # tiles go in (j, j+4) pairs so combine j unlocks early; on
                # i=1 phases each combine is followed by the out-DMA. The
                # very last phase runs `fine` (per-batch-row evac/combine/
                # DMA) so the kernel tail after the final matmul is short.
                o = OFF[d]
                bsl = slice(2 * h, 2 * h + BH)
                for j in range(HB):
                    pair = {}
                    for mb, fn, tag in ((j, AF.Relu, "nl"),
                                        (j + MB // 2, AF.Sigmoid, "g")):
                        ps = ps_pool.tile([P, BH, S], f32, tag="ps", name="ps")
                        dst = evac_pool.tile([P, BH, S], f16, tag=tag, name=tag)
                        for bh in range(BH):
                            b = 2 * h + bh
                            for kb in range(HB):
                                nc.tensor.matmul(
                                    ps[:, bh, :],
                                    lhsT=wt_sb[:, d, i, kb,
                                               mb * P:(mb + 1) * P],
                                    rhs=xin[:, kb, b, o:o + S],
                                    start=(kb == 0),
                                    stop=(kb == HB - 1),
                                )
                            if fine:
                                nc.scalar.activation(
                                    out=dst[:, bh, :], in_=ps[:, bh, :],
                                    func=fn,
                                    bias=hwb_sb[:, l, d, i, mb:mb + 1],
                                )
                        if not fine:
                            nc.scalar.activation(
                                out=dst, in_=ps,
                                func=fn,
                                bias=hwb_sb[:, l, d, i, mb:mb + 1],
                            )
                        pair[tag] = dst
                    nl, g = pair["nl"], pair["g"]
                    # combine: xout = ((xin - nl) * g) + nl on DVE or Pool
                    eng = nc.gpsimd if j in pool_js else nc.vector
                    hoff = 0 if d == 0 else H
                    if fine:
                        for bh in range(BH):
                            b = 2 * h + bh
                            xi = xin[:, j, b, o:o + S]
                            xo = xout[:, j, b, o:o + S]
                            eng.tensor_sub(xo, xi, nl[:, bh, :])
                            eng.tensor_mul(xo, xo, g[:, bh, :])
                            eng.tensor_add(xo, xo, nl[:, bh, :])
                            ov = out[l, b][hoff + j * P:hoff + (j + 1) * P, :]
                            nc.sync.dma_start(out=ov, in_=xo)
                        continue
                    xi = xin[:, j, bsl, o:o + S]
                    xo = xout[:, j, bsl, o:o + S]
                    if defer_dve_combines and j not in pool_js:
                        # park this DVE combine; it is emitted a few phases
                        # later so the conv chains queued on DVE behind it
                        # are not delayed (consumers are far downstream)
                        deferred_combines.append((xo, xi, nl, g))
                        continue
                    eng.tensor_sub(xo, xi, nl)
                    eng.tensor_mul(xo, xo, g)
                    eng.tensor_add(xo, xo, nl)
                    if i == NHW - 1:
                        for bh in range(BH):
                            b = 2 * h + bh
                            ov = out[l, b][hoff + j * P:hoff + (j + 1) * P, :]
                            nc.sync.dma_start(
                                out=ov, in_=xout[:, j, b, o:o + S]
                            )
            # combine->Pool assignment. Constraints: i=1 combines feed the
            # next-layer conv chains on DVE, so they must stay on DVE (a
            # Pool combine would stall the DVE queue behind the waiting
            # conv chain); the startup conv stretch offloads its combines
            # to Pool; the final phase stays on DVE for a fast tail (Pool
            # combines are 3.6x slower per op).
            def pool_js_for(l, i, d, h):
                if l == 0 and i == 0:
                    return (0, 1, 2, 3) if d == 0 else (1, 3)
                if l == 0 and i == 1:
                    return (1, 3)
                if l == 1 and i == 0:
                    return (1,)
                if (d, h) == (1, 1):
                    return ()
                return (1, 3)

            for l in range(L):
                if l == 0:
                    for d in range(2):
                        for h in range(2):
                            for hb in range(HB):
                                conv_chain(0, d, h, hb, rot[d][0], rot[d][1],
                                           k0_act=True)
                    # pads for layer 1 go into this layer's conv-output
                    # buffer (= conv(l+1)'s source); its pad slot is
                    # untouched by this layer's writes
                    for d in range(2):
                        load_pads(1, rot[d][1], d)
                for i in range(NHW):
                    # pending next-layer conv-chain groups: each (d, h)
                    # group is emitted one phase AFTER its combines, so
                    # later phases' combines interleave between chain
                    # groups on DVE instead of queueing behind them all
                    for d in (0, 1):
                        a, b_, c = rot[d]
                        for h in range(2):
                            if i == 0:
                                gemm_phase(l, d, 0, h, b_, c,
                                           pool_js_for(l, 0, d, h), None,
                                           defer_dve_combines=(
                                               l == 0 and d == 1))
                            else:
                                gemm_phase(l, d, 1, h, c, b_,
                                           pool_js_for(l, 1, d, h), None,
                                           fine=(l == L - 1 and d == 1
                                                 and h == 1))
                                if deferred_combines:
                                    # flush combines parked during the
                                    # conv-gated i=0 stretch (consumers
                                    # are still phases away)
                                    for xo, xi, nl_, g_ in deferred_combines:
                                        nc.vector.tensor_sub(xo, xi, nl_)
                                        nc.vector.tensor_mul(xo, xo, g_)
                                        nc.vector.tensor_add(xo, xo, nl_)
                                    deferred_combines.clear()
                                if l + 1 < L:
                                    for j in range(HB):
                                        conv_chain(l + 1, d, h, j, b_, a)
                    if l + 1 < L:
                        # stream next layer's weights for this i as soon as
                        # layer l's phases stop reading the slot
                        for d in range(2):
                            load_wt(l + 1, d, i)

                for d in range(2):
                    a, b_, c = rot[d]
                    rot[d] = [b_, a, c]
    nc.finalize()
    return nc


def _get_nc():
    if "nc" not in _CACHE:
        _CACHE["nc"] = _build_nc()
    return _CACHE["nc"]


def _prep_shared(inputs):
    fwd_pads = np.asarray(inputs["fwd_pads"], np.float32)   # [L, W, H]
    bwd_pads = np.asarray(inputs["bwd_pads"], np.float32)
    fwd_ws = np.asarray(inputs["fwd_ws"], np.float32)       # [L, W+1]
    bwd_ws = np.asarray(inputs["bwd_ws"], np.float32)
    fwd_hw_W = np.asarray(inputs["fwd_hw_W"], np.float32)   # [L, NHW, 2H, H]
    fwd_hw_b = np.asarray(inputs["fwd_hw_b"], np.float32)   # [L, NHW, 2H]
    bwd_hw_W = np.asarray(inputs["bwd_hw_W"], np.float32)
    bwd_hw_b = np.asarray(inputs["bwd_hw_b"], np.float32)

    wt = np.empty((L, 2, NHW, H, 2 * H), np.float16)
    hwb = np.empty((L, 2, NHW, P, MB), np.float32)
    for l in range(L):
        for i in range(NHW):
            wt[l, 0, i] = fwd_hw_W[l, i].T
            wt[l, 1, i] = bwd_hw_W[l, i].T
            hwb[l, 0, i] = fwd_hw_b[l, i].reshape(MB, P).T
            hwb[l, 1, i] = bwd_hw_b[l, i].reshape(MB, P).T

    ws = np.stack([fwd_ws, bwd_ws], axis=1)              # [L, 2, W+1]

    return {
        "ws": np.ascontiguousarray(ws),
        "wt": np.ascontiguousarray(wt),
        "padl": np.ascontiguousarray(
            fwd_pads.transpose(0, 2, 1).astype(np.float16)),   # [L, H, W]
        "padr": np.ascontiguousarray(
            bwd_pads.transpose(0, 2, 1).astype(np.float16)),
        "hwb": np.ascontiguousarray(hwb),
    }


def kernel(**inputs) -> np.ndarray:
    from concourse.bass_utils import run_bass_kernel_spmd

    x = np.asarray(inputs["inputs"], np.float32)            # [B, S, H]
    xt = np.ascontiguousarray(
        x.transpose(0, 2, 1).astype(np.float16))            # [B, H, S] f16
    shared = _prep_shared(inputs)

    nc = _get_nc()
    in_maps = []
    for c in range(NCORES):
        m = dict(shared)
        m["xt"] = np.ascontiguousarray(xt[c * BL:(c + 1) * BL])
        in_maps.append(m)
    res = run_bass_kernel_spmd(nc, in_maps, core_ids=list(range(NCORES)))
    _CACHE["last_res"] = res
    outs = [r["out"] for r in res.results]                  # [L, BL, 2H, S] f16
    full = np.concatenate(outs, axis=1)                     # [L, B, 2H, S]
    return np.ascontiguousarray(
        full.transpose(0, 1, 3, 2)).astype(np.float32)      # [L, B, S, 2H]
